# revision 1
# baseline (speedup 1.0000x reference)
"""GAT (2-layer, PyG-style) distributed Bass kernel for 8 Trainium2 NeuronCores.

Strategy (graph/data parallel, per sharding hint):
  - Nodes are partitioned into 8 contiguous blocks; core c owns destination
    nodes [c*N/8, (c+1)*N/8) and all edges incident to them (plus self loops).
  - Layer 1: every core builds the full node feature table
    xh1ext = x @ [W1 | W1@a_src_bd] (redundant compute is cheaper than
    cross-core collectives here), then processes its destination tiles:
    a hardware dma_gather fetches per-edge source rows [xh(128) | e_src(8)],
    attention coefficients are formed with leaky_relu+exp (no max-subtraction
    needed: scores are O(1) so exp never overflows; softmax is exactly
    equivalent), and a 0/1 selection-matrix matmul on the tensor engine
    performs the per-destination segment reduction of [msg | ea] in PSUM.
  - Host reassembles the transposed hidden table h_T from the 8 shards
    (pure data movement), then launch 2 repeats the same structure with
    42-wide features for the single-head output layer.

SPMD constraints force fully uniform static structure across cores: every
(dst-tile x src-quarter) edge segment is padded to S chunks of 128 edges
(pad edges gather row 0 and use an out-of-range dst slot so selection
matrices zero them out). Source indices are split into 4 quarters because
dma_gather indices are int16.
"""

import math
import os
import sys

for _p in ("/opt/trn_rl_repo", "/root/.axon_site/_ro/trn_rl_repo"):
    if os.path.isdir(_p) and _p not in sys.path:
        sys.path.insert(0, _p)

import numpy as np
import ml_dtypes
from contextlib import ExitStack

import concourse.bacc as bacc
import concourse.bass as bass
import concourse.tile as tile
from concourse import mybir
from concourse.bass_utils import run_bass_kernel_spmd

F32 = mybir.dt.float32
BF16 = mybir.dt.bfloat16
I16 = mybir.dt.int16
AF = mybir.ActivationFunctionType
ALU = mybir.AluOpType

NEG_SLOPE = 0.2
PSUM_PP_BUFS = 1
EPS = 1e-16
P = 128
PAD_DST = 200.0  # sentinel dst_local for pad edges; never matches iota 0..127


# --------------------------------------------------------------------------
# host-side graph preprocessing
# --------------------------------------------------------------------------

def _round_up(a, b):
    return (a + b - 1) // b * b


class EdgeStruct:
    """Uniform SPMD edge layout shared by both layers."""

    def __init__(self, src, dst, N, n_cores, G=3):
        self.N = N
        self.n_cores = n_cores
        self.G = G
        self.Npad = _round_up(N, 512)
        self.Qsz = self.Npad // 4
        assert self.Qsz <= 32767
        assert N % n_cores == 0
        self.npc = N // n_cores                      # dst nodes per core
        self.T = math.ceil(self.npc / P)             # real dst tiles per core
        self.T_pad = _round_up(self.T, G)
        self.n_groups = self.T_pad // G
        nseg = self.T_pad * 4

        src = src.astype(np.int64)
        dst = dst.astype(np.int64)

        per_core = []
        max_cnt = 0
        for c in range(n_cores):
            lo = c * self.npc
            sel = (dst >= lo) & (dst < lo + self.npc)
            s_c = src[sel]
            dl = dst[sel] - lo                        # local dst id
            t_all = dl >> 7                           # dst tile
            q_all = s_c // self.Qsz                   # src quarter
            key = t_all * 4 + q_all
            order = np.argsort(key, kind="stable")
            s_c, dl, key = s_c[order], dl[order], key[order]
            cnt = np.bincount(key, minlength=nseg)
            max_cnt = max(max_cnt, int(cnt.max()))
            per_core.append((s_c, dl, key, cnt))

        self.S = max(1, math.ceil(max_cnt / P))      # chunks per segment
        S, G_, Qsz = self.S, G, self.Qsz
        self.ncols = 4 * G * S                       # chunk columns per group
        assert self.ncols <= P, f"ncols={self.ncols} > 128; lower S or G"
        slots_seg = S * P

        self.gidx = []    # [n_groups*4*128, G*S*8] int16
        self.gdl = []     # [n_groups*128, ncols]   bf16
        self.gdr = []     # [n_groups*128, 128]     bf16
        for c in range(n_cores):
            s_c, dl, key, cnt = per_core[c]
            flat_idx = np.zeros(nseg * slots_seg, np.int16)
            flat_dl = np.full(nseg * slots_seg, PAD_DST, np.float32)
            starts = np.concatenate([[0], np.cumsum(cnt)])[:-1]
            # position of each edge inside the padded segment layout
            pos_in_seg = np.arange(len(s_c)) - starts[key]
            base = key * slots_seg
            pos = base + pos_in_seg
            q_of_edge = key % 4
            flat_idx[pos] = (s_c - q_of_edge * Qsz).astype(np.int16)
            flat_dl[pos] = (dl & 127).astype(np.float32)

            # flat layout is segment-major: seg = t*4+q, inside: s*128+p.
            # regroup to gather order: per (g, q): (t_loc, s, p)
            fi = flat_idx.reshape(self.T_pad, 4, S, P)
            fd = flat_dl.reshape(self.T_pad, 4, S, P)
            # -> [n_groups, G, 4, S, P] -> [n_groups, 4, G, S, P]
            fi = fi.reshape(self.n_groups, G_, 4, S, P).transpose(0, 2, 1, 3, 4)
            fd = fd.reshape(self.n_groups, G_, 4, S, P).transpose(0, 2, 1, 3, 4)

            # gather idx arrays: flat i = (t_loc*S+s)*128+p ; wrapped [128, i/16]
            fi2 = fi.reshape(self.n_groups, 4, G_ * S * P)
            w = fi2.reshape(self.n_groups, 4, G_ * S * 8, 16)
            w = np.transpose(w, (0, 1, 3, 2))              # [g, 4, 16, cols16]
            w = np.tile(w, (1, 1, 8, 1))                   # replicate to 128
            self.gidx.append(
                np.ascontiguousarray(w.reshape(self.n_groups * 4 * P, G_ * S * 8))
            )

            # dst_local in both layouts; group buffer col = q*(G*S)+t_loc*S+s
            fcol = fd.reshape(self.n_groups, self.ncols, P)   # [g, c, p]
            gdr = np.zeros((self.n_groups, P, P), np.float32)
            gdr[:, : self.ncols, :] = fcol
            gdl = np.transpose(fcol, (0, 2, 1))               # [g, p, c]
            self.gdl.append(
                np.ascontiguousarray(
                    gdl.reshape(self.n_groups * P, self.ncols)
                ).astype(ml_dtypes.bfloat16)
            )
            self.gdr.append(
                np.ascontiguousarray(gdr.reshape(self.n_groups * P, P)).astype(
                    ml_dtypes.bfloat16
                )
            )


# --------------------------------------------------------------------------
# device kernel builder (shared by both layers)
# --------------------------------------------------------------------------

def build_layer_kernel(es: EdgeStruct, layer: int):
    """layer 1: feat table row [xh1(128)|e_src1(8)|junk], elem 192 f32,
               heads=8, csz=16, epilogue = softmax-div + ELU + transpose out.
       layer 2: row [xh2(40)|e_src2(1)|junk], elem 64 f32, heads=1, csz=40,
               epilogue = softmax-div, row-major out."""
    Npad, T_pad, G, S, ncols = es.Npad, es.T_pad, es.G, es.S, es.ncols
    n_groups, Qsz = es.n_groups, es.Qsz
    if layer == 1:
        ELEM, H, CSZ, WCOLS = 192, 8, 16, 136
    else:
        ELEM, H, CSZ, WCOLS = 64, 1, 40, 41
    # self-loop edges are not in the edge lists; their contribution is added
    # analytically in the tile epilogue from the core's own-node rows.
    MW = H * CSZ                      # message width (128 / 40)
    AW = MW + H                       # [msg | ea] width (136 / 41)

    nc = bacc.Bacc("TRN2", target_bir_lowering=False, debug=False,
                   num_devices=es.n_cores)
    ap = {}
    ap["xT"] = nc.dram_tensor("xT", [P, Npad], F32, kind="ExternalInput").ap()
    ap["xTm"] = nc.dram_tensor("xTm", [P, T_pad * P], F32,
                               kind="ExternalInput").ap()
    ap["wext"] = nc.dram_tensor("wext", [P, WCOLS], F32,
                                kind="ExternalInput").ap()
    ap["brow"] = nc.dram_tensor("brow", [1, WCOLS], F32,
                                kind="ExternalInput").ap()
    ap["ones_f"] = nc.dram_tensor("ones_f", [1, P], F32,
                                  kind="ExternalInput").ap()
    ap["wdst"] = nc.dram_tensor("wdst", [P, H], F32, kind="ExternalInput").ap()
    ap["gidx"] = nc.dram_tensor("gidx", [n_groups * 4 * P, G * S * 8], I16,
                                kind="ExternalInput").ap()
    ap["gdl"] = nc.dram_tensor("gdl", [n_groups * P, ncols], BF16,
                               kind="ExternalInput").ap()
    ap["gdr"] = nc.dram_tensor("gdr", [n_groups * P, P], BF16,
                               kind="ExternalInput").ap()
    ap["iota_bf"] = nc.dram_tensor("iota_bf", [P, P], BF16,
                                   kind="ExternalInput").ap()
    ap["iota_col"] = nc.dram_tensor("iota_col", [P, 1], F32,
                                    kind="ExternalInput").ap()
    ap["ones_bf"] = nc.dram_tensor("ones_bf", [1, P], BF16,
                                   kind="ExternalInput").ap()
    ap["idn"] = nc.dram_tensor("idn", [P, P], F32, kind="ExternalInput").ap()
    if layer == 1:
        out_ap = nc.dram_tensor("hT", [P, T_pad * P], F32,
                                kind="ExternalOutput").ap()
    else:
        out_ap = nc.dram_tensor("logits", [T_pad * P, CSZ], F32,
                                kind="ExternalOutput").ap()
    tbl = nc.dram_tensor("tbl", [Npad, ELEM], F32, kind="Internal").ap()
    own_tbl = nc.dram_tensor("own_tbl", [T_pad * P, WCOLS], F32,
                             kind="Internal").ap()

    with tile.TileContext(nc) as tc, ExitStack() as ctx:
        cpool = ctx.enter_context(tc.tile_pool(name="consts", bufs=1))

        # ---- constants ----
        wext = cpool.tile([P, WCOLS], F32)
        nc.sync.dma_start(wext[:], ap["wext"])
        brow = cpool.tile([1, WCOLS], F32)
        nc.sync.dma_start(brow[:], ap["brow"])
        ones_f = cpool.tile([1, P], F32)
        nc.sync.dma_start(ones_f[:], ap["ones_f"])
        wdst = cpool.tile([P, H], F32)
        nc.sync.dma_start(wdst[:], ap["wdst"])
        iota_bf = cpool.tile([P, P], BF16)
        nc.sync.dma_start(iota_bf[:], ap["iota_bf"])
        iota_col = cpool.tile([P, 1], F32)
        nc.sync.dma_start(iota_col[:], ap["iota_col"])
        ones_bf = cpool.tile([1, P], BF16)
        nc.sync.dma_start(ones_bf[:], ap["ones_bf"])
        idn = cpool.tile([P, P], F32)
        nc.sync.dma_start(idn[:], ap["idn"])
        edst_sb = cpool.tile([P, T_pad * H], F32)

        with tc.tile_pool(name="pre_sb", bufs=4) as psb, \
                tc.tile_pool(name="pre_ps", bufs=2, space="PSUM") as pps:
            # ---- pre-pass A: full feature table ----
            for i in range(Npad // P):
                xt = psb.tile([P, P], F32, tag="xt")
                nc.sync.dma_start(xt[:], ap["xT"][:, i * P:(i + 1) * P])
                ppt = pps.tile([P, WCOLS], F32, tag="ppt")
                nc.tensor.matmul(out=ppt[:], lhsT=xt[:], rhs=wext[:],
                                 start=True, stop=False, skip_group_check=True)
                # bias row: feature-table rows get +bias (attention-score
                # columns of brow are zero); since sum(att)=1 per dst this
                # reproduces "+ bias" after aggregation.
                nc.tensor.matmul(out=ppt[:], lhsT=ones_f[:], rhs=brow[:],
                                 start=False, stop=True, skip_group_check=True)
                ot = psb.tile([P, WCOLS], F32, tag="ot")
                nc.vector.tensor_copy(out=ot[:], in_=ppt[:])
                nc.sync.dma_start(tbl[i * P:(i + 1) * P, 0:WCOLS], ot[:])

            # ---- pre-pass B: own-node rows [xh+b|e_src] (DRAM) and e_dst
            # (SBUF-resident), for e_dst matmuls and self-loop epilogue ----
            for t in range(T_pad):
                xt = psb.tile([P, P], F32, tag="xt2")
                nc.sync.dma_start(xt[:], ap["xTm"][:, t * P:(t + 1) * P])
                po = pps.tile([P, WCOLS], F32, tag="po")
                nc.tensor.matmul(out=po[:], lhsT=xt[:], rhs=wext[:],
                                 start=True, stop=False, skip_group_check=True)
                nc.tensor.matmul(out=po[:], lhsT=ones_f[:], rhs=brow[:],
                                 start=False, stop=True, skip_group_check=True)
                oo = psb.tile([P, WCOLS], F32, tag="oo")
                nc.vector.tensor_copy(out=oo[:], in_=po[:])
                nc.sync.dma_start(own_tbl[t * P:(t + 1) * P, :], oo[:])
                pe = pps.tile([P, H], F32, tag="pe")
                nc.tensor.matmul(out=pe[:], lhsT=xt[:], rhs=wdst[:],
                                 start=True, stop=True)
                nc.vector.tensor_copy(out=edst_sb[:, t * H:(t + 1) * H],
                                      in_=pe[:])

        # ---- edge pass ----
        sb = ctx.enter_context(tc.tile_pool(name="sb", bufs=3))
        gbp = ctx.enter_context(tc.tile_pool(name="gbuf", bufs=2))
        pp = ctx.enter_context(tc.tile_pool(name="pp", bufs=PSUM_PP_BUFS, space="PSUM"))
        ptp = ctx.enter_context(tc.tile_pool(name="ptp", bufs=1,
                                             space="PSUM"))
        pacc = ctx.enter_context(tc.tile_pool(name="pacc", bufs=min(G, 2),
                                              space="PSUM"))
        GSP = G * S * P
        tc.strict_bb_all_engine_barrier()
        for g in range(n_groups):
            if g % 2 == 0:
                tc.strict_bb_all_engine_barrier()
            gb = gbp.tile([P, ncols * ELEM], F32, tag="gb")
            gb3 = gb[:].rearrange("p (c k) -> p c k", k=ELEM)
            idxs = sb.tile([P, 4 * G * S * 8], I16, tag="idx")
            for q in range(4):
                nc.sync.dma_start(
                    idxs[:, q * G * S * 8:(q + 1) * G * S * 8],
                    ap["gidx"][(g * 4 + q) * P:(g * 4 + q + 1) * P, :])
            dlt = sb.tile([P, ncols], BF16, tag="dl")
            nc.sync.dma_start(dlt[:], ap["gdl"][g * P:(g + 1) * P, :])
            drt = sb.tile([P, P], BF16, tag="dr")
            nc.sync.dma_start(drt[:], ap["gdr"][g * P:(g + 1) * P, :])
            MAXC = 4  # sub-gather size in 128-idx chunks (HW-validated regime)
            for q in range(4):
                for c0 in range(0, G * S, MAXC):
                    c1 = min(c0 + MAXC, G * S)
                    nc.gpsimd.dma_gather(
                        out_ap=gb3[:, q * G * S + c0:q * G * S + c1, :],
                        in_ap=tbl[q * Qsz:(q + 1) * Qsz, :],
                        idxs_ap=idxs[:, (q * G * S + c0) * 8:
                                     (q * G * S + c1) * 8],
                        num_idxs=(c1 - c0) * P,
                        num_idxs_reg=(c1 - c0) * P,
                        elem_size=ELEM,
                    )
            for t_loc in range(G):
                t = g * G + t_loc
                acc = pacc.tile([P, AW], F32, tag="acc")
                for q in range(4):
                    for s in range(S):
                        c = q * G * S + t_loc * S + s
                        xh_ch = gb[:, c * ELEM:c * ELEM + MW]
                        es_ch = gb[:, c * ELEM + MW:c * ELEM + MW + H]
                        # S_eT[e,d] = (dst_local[e] == d)
                        seT = sb.tile([P, P], F32, tag="seT")
                        nc.vector.tensor_tensor(
                            out=seT[:],
                            in0=dlt[:, c:c + 1].to_broadcast([P, P]),
                            in1=iota_bf[:], op=ALU.is_equal)
                        # S_dT = transpose(S_eT)
                        bc = pp.tile([P, P], F32, tag="bc")
                        nc.tensor.transpose(out=bc[:], in_=seT[:],
                                            identity=idn[:])
                        sdT = sb.tile([P, P], F32, tag="sdT")
                        nc.vector.tensor_copy(out=sdT[:], in_=bc[:])
                        # e_dst per edge: S_dT.T @ e_dst_tile -> [e, H]
                        ed = pp.tile([P, H], F32, tag="ed")
                        nc.tensor.matmul(
                            out=ed[:], lhsT=sdT[:],
                            rhs=edst_sb[:, t * H:(t + 1) * H],
                            start=True, stop=True)
                        mea = sb.tile([P, AW], F32, tag="mea")
                        al = sb.tile([P, H], F32, tag="al")
                        al2 = sb.tile([P, H], F32, tag="al2")
                        nc.vector.tensor_tensor(out=al[:], in0=es_ch,
                                                in1=ed[:], op=ALU.add)
                        # leaky_relu(a) = max(a, slope*a)
                        nc.vector.tensor_scalar_mul(out=al2[:], in0=al[:],
                                                    scalar1=NEG_SLOPE)
                        nc.vector.tensor_tensor(out=al[:], in0=al[:],
                                                in1=al2[:], op=ALU.max)
                        nc.scalar.activation(out=mea[:, MW:AW], in_=al[:],
                                             func=AF.Exp)
                        # msg = xh * ea (broadcast over channel group)
                        if H == 1:
                            nc.vector.tensor_tensor(
                                out=mea[:, 0:MW],
                                in0=mea[:, MW:AW].to_broadcast([P, MW]),
                                in1=xh_ch, op=ALU.mult)
                        else:
                            ea3 = mea[:, MW:AW].rearrange(
                                "p (h o) -> p h o", o=1).to_broadcast(
                                [P, H, CSZ])
                            xh3 = xh_ch.rearrange("p (h c) -> p h c", c=CSZ)
                            mea3 = mea[:, 0:MW].rearrange(
                                "p (h c) -> p h c", c=CSZ)
                            nc.vector.tensor_tensor(out=mea3, in0=ea3,
                                                    in1=xh3, op=ALU.mult)
                        # segment-reduce into the tile accumulator
                        nc.tensor.matmul(out=acc[:], lhsT=seT[:], rhs=mea[:],
                                         start=(q == 0 and s == 0),
                                         stop=(q == 3 and s == S - 1),
                                         skip_group_check=True)
                # ---- tile epilogue (adds analytic self-loop term) ----
                own = sb.tile([P, WCOLS], F32, tag="own")
                nc.sync.dma_start(own[:], own_tbl[t * P:(t + 1) * P, :])
                als = sb.tile([P, H], F32, tag="als")
                als2 = sb.tile([P, H], F32, tag="als2")
                nc.vector.tensor_tensor(out=als[:], in0=own[:, MW:WCOLS],
                                        in1=edst_sb[:, t * H:(t + 1) * H],
                                        op=ALU.add)
                nc.vector.tensor_scalar_mul(out=als2[:], in0=als[:],
                                            scalar1=NEG_SLOPE)
                nc.vector.tensor_tensor(out=als[:], in0=als[:], in1=als2[:],
                                        op=ALU.max)
                eas = sb.tile([P, H], F32, tag="eas")
                nc.scalar.activation(out=eas[:], in_=als[:], func=AF.Exp)
                # self message: note own xh columns include +bias, matching
                # the gathered table rows.
                smsg = sb.tile([P, MW], F32, tag="smsg")
                if H == 1:
                    nc.vector.tensor_tensor(
                        out=smsg[:], in0=eas[:, 0:1].to_broadcast([P, MW]),
                        in1=own[:, 0:MW], op=ALU.mult)
                else:
                    nc.vector.tensor_tensor(
                        out=smsg[:].rearrange("p (h c) -> p h c", c=CSZ),
                        in0=eas[:].rearrange("p (h o) -> p h o", o=1)
                        .to_broadcast([P, H, CSZ]),
                        in1=own[:, 0:MW].rearrange("p (h c) -> p h c", c=CSZ),
                        op=ALU.mult)
                unorm = sb.tile([P, MW], F32, tag="unorm")
                nc.vector.tensor_tensor(out=unorm[:], in0=acc[:, 0:MW],
                                        in1=smsg[:], op=ALU.add)
                den = sb.tile([P, H], F32, tag="den")
                nc.vector.tensor_tensor(out=den[:], in0=acc[:, MW:AW],
                                        in1=eas[:], op=ALU.add)
                nc.vector.tensor_scalar_add(out=den[:], in0=den[:],
                                            scalar1=EPS)
                rec = sb.tile([P, H], F32, tag="rec")
                nc.vector.reciprocal(out=rec[:], in_=den[:])
                otile = sb.tile([P, MW], F32, tag="otile")
                if H == 1:
                    nc.vector.tensor_tensor(
                        out=otile[:], in0=rec[:, 0:1].to_broadcast([P, MW]),
                        in1=unorm[:], op=ALU.mult)
                else:
                    rec3 = rec[:].rearrange("p (h o) -> p h o", o=1) \
                        .to_broadcast([P, H, CSZ])
                    acc3 = unorm[:].rearrange("p (h c) -> p h c", c=CSZ)
                    ot3 = otile[:].rearrange("p (h c) -> p h c", c=CSZ)
                    nc.vector.tensor_tensor(out=ot3, in0=rec3, in1=acc3,
                                            op=ALU.mult)
                if layer == 1:
                    # ELU then transpose out
                    tmp = sb.tile([P, MW], F32, tag="tmp")
                    nc.vector.tensor_scalar_min(out=tmp[:], in0=otile[:],
                                                scalar1=0.0)
                    nc.scalar.activation(out=tmp[:], in_=tmp[:], func=AF.Exp)
                    nc.scalar.activation(out=otile[:], in_=otile[:],
                                         func=AF.Relu)
                    nc.vector.tensor_tensor(out=otile[:], in0=tmp[:],
                                            in1=otile[:], op=ALU.add)
                    nc.vector.tensor_scalar_add(out=otile[:], in0=otile[:],
                                                scalar1=-1.0)
                    tp = ptp.tile([P, P], F32, tag="tp")
                    nc.tensor.transpose(out=tp[:], in_=otile[:],
                                        identity=idn[:])
                    hTt = sb.tile([P, P], F32, tag="hTt")
                    nc.vector.tensor_copy(out=hTt[:], in_=tp[:])
                    nc.sync.dma_start(out_ap[:, t * P:(t + 1) * P], hTt[:])
                else:
                    nc.sync.dma_start(out_ap[t * P:(t + 1) * P, :], otile[:])

    nc.compile()
    return nc


# --------------------------------------------------------------------------
# host orchestration
# --------------------------------------------------------------------------

def _consts_inputs():
    iota = np.arange(P, dtype=np.float32)
    return {
        "iota_bf": np.tile(iota.astype(ml_dtypes.bfloat16)[None, :], (P, 1)),
        "iota_col": iota[:, None].copy(),
        "ones_bf": np.ones((1, P), ml_dtypes.bfloat16),
        "ones_f": np.ones((1, P), np.float32),
        "idn": np.eye(P, dtype=np.float32),
    }


def _blockdiag(att):
    """[H, C] attention vector -> [H*C, H] block-diagonal matrix."""
    H, C = att.shape
    out = np.zeros((H * C, H), np.float32)
    for h in range(H):
        out[h * C:(h + 1) * C, h] = att[h]
    return out


def run_gat(x, edge_index, W1, att_src1, att_dst1, b1, W2, att_src2, att_dst2,
            b2, N, n_cores, G=2, es=None, verbose=False):
    x = np.asarray(x, np.float32)
    src = np.asarray(edge_index[0]).astype(np.int64)
    dst = np.asarray(edge_index[1]).astype(np.int64)
    # self-loops are handled analytically inside the kernel epilogue

    if es is None:
        es = EdgeStruct(src, dst, N, n_cores, G=G)
    npc, Npad, T_pad = es.npc, es.Npad, es.T_pad

    consts = _consts_inputs()
    xT = np.zeros((P, Npad), np.float32)
    xT[:, :N] = np.asarray(x, np.float32).T

    W1 = np.asarray(W1, np.float32)
    w1ext = np.concatenate(
        [W1, W1 @ _blockdiag(np.asarray(att_src1, np.float32))], axis=1)
    w1dst = W1 @ _blockdiag(np.asarray(att_dst1, np.float32))
    brow1 = np.zeros((1, w1ext.shape[1]), np.float32)
    brow1[0, :128] = np.asarray(b1, np.float32)

    nc1 = build_layer_kernel(es, 1)
    in_maps = []
    for c in range(n_cores):
        xTm = np.zeros((P, T_pad * P), np.float32)
        xTm[:, :npc] = xT[:, c * npc:(c + 1) * npc]
        in_maps.append({
            "xT": xT, "xTm": xTm, "wext": w1ext, "wdst": w1dst,
            "brow": brow1,
            "gidx": es.gidx[c], "gdl": es.gdl[c], "gdr": es.gdr[c],
            **consts,
        })
    res1 = run_bass_kernel_spmd(nc1, in_maps, core_ids=list(range(n_cores)))
    hT = np.zeros((P, Npad), np.float32)
    for c in range(n_cores):
        hT[:, c * npc:(c + 1) * npc] = res1.results[c]["hT"][:, :npc]

    W2 = np.asarray(W2, np.float32)
    w2ext = np.concatenate(
        [W2, W2 @ _blockdiag(np.asarray(att_src2, np.float32))], axis=1)
    w2dst = W2 @ _blockdiag(np.asarray(att_dst2, np.float32))
    brow2 = np.zeros((1, w2ext.shape[1]), np.float32)
    brow2[0, :40] = np.asarray(b2, np.float32)

    nc2 = build_layer_kernel(es, 2)
    in_maps2 = []
    for c in range(n_cores):
        hTm = np.zeros((P, T_pad * P), np.float32)
        hTm[:, :npc] = hT[:, c * npc:(c + 1) * npc]
        in_maps2.append({
            "xT": hT, "xTm": hTm, "wext": w2ext, "wdst": w2dst,
            "brow": brow2,
            "gidx": es.gidx[c], "gdl": es.gdl[c], "gdr": es.gdr[c],
            **consts,
        })
    res2 = run_bass_kernel_spmd(nc2, in_maps2, core_ids=list(range(n_cores)))
    out = np.zeros((N, 40), np.float32)
    for c in range(n_cores):
        out[c * npc:(c + 1) * npc] = res2.results[c]["logits"][:npc, :]
    return out


def kernel(x, edge_index, W1, att_src1, att_dst1, b1, W2, att_src2, att_dst2,
           b2):
    N = int(np.asarray(x).shape[0])
    return run_gat(x, edge_index, W1, att_src1, att_dst1, b1, W2, att_src2,
                   att_dst2, b2, N=N, n_cores=8)



# revision 13
# speedup vs baseline: 2.5538x; 2.5538x over previous
"""GAT (2-layer, PyG-style) distributed Bass kernel for 8 Trainium2 NeuronCores.

Strategy (graph/data parallel per sharding hint), v2 — engine-balanced rewrite:
  - Host balances destination nodes across cores and across (tile, src-quarter)
    cells so each segment needs only S=4 chunks of 128 edges (2% pad).
  - Table rows are bf16 with head-interleaved features [xh(cc,h) | e_src]:
    the per-edge message multiply hits the DVE 2x packed mode, and the
    attention-score adds/exp are batched per group on DVE+ACT.
  - Edge-major one-hot (acc matmul lhsT) is built on DVE with a single
    tensor_scalar(is_equal) using a per-partition scalar; dst-major one-hot
    (e_dst matmul lhsT) is streamed from host as fp8 (exact 0/1).
  - Layer 2 (H=1) folds the attention weight into the one-hot with a fused
    is_equal+mult tensor_scalar, and a constant-1 table column makes the
    softmax denominator ride the same accumulation matmul.
  - Biases are folded algebraically (x-shift through W; rank-1 correction on
    e_dst), so the feature-table prepass is one matmul per node tile.
"""

import math
import os
import sys

for _p in ("/opt/trn_rl_repo", "/root/.axon_site/_ro/trn_rl_repo"):
    if os.path.isdir(_p) and _p not in sys.path:
        sys.path.insert(0, _p)

import numpy as np
import ml_dtypes
from contextlib import ExitStack

import concourse.bacc as bacc
import concourse.bass as bass
import concourse.tile as tile
from concourse import mybir
from concourse.bass_utils import run_bass_kernel_spmd

F32 = mybir.dt.float32
BF16 = mybir.dt.bfloat16
FP8 = mybir.dt.float8e4
I16 = mybir.dt.int16
AF = mybir.ActivationFunctionType
ALU = mybir.AluOpType
NPBF = ml_dtypes.bfloat16

P = 128
NEG_SLOPE = 0.2
PAD_DST = 200.0
MAXC = int(os.environ.get("GAT_MAXC", "8"))  # chunks per dma_gather call
SDT_BF16 = os.environ.get("GAT_SDT_BF16", "0") == "1"


def _round_up(a, b):
    return (a + b - 1) // b * b


# --------------------------------------------------------------------------
# host-side graph preprocessing
# --------------------------------------------------------------------------

class EdgeStruct:
    """Balanced SPMD edge layout shared by both layers."""

    def __init__(self, src, dst, N, n_cores, G=4, tiles_slack=2):
        self.N = N
        self.n_cores = n_cores
        self.G = G
        self.Npad = _round_up(N, 512)
        self.Qsz = self.Npad // 4
        assert self.Qsz <= 32767
        assert N % n_cores == 0
        self.npc = N // n_cores
        T = _round_up(math.ceil(self.npc / P) + tiles_slack, G)
        self.T = T
        self.n_groups = T // G

        src = np.asarray(src, np.int64)
        dst = np.asarray(dst, np.int64)
        E = len(src)
        q_of = src // self.Qsz
        # per-node degree 4-vector over src quarters
        deg4 = np.bincount(dst * 4 + q_of, minlength=N * 4).reshape(N, 4)
        tot = deg4.sum(1)

        # ---- balance nodes across cores (serpentine on degree order) ----
        order = np.argsort(-tot, kind="stable")
        core_of = np.full(N, -1, np.int32)
        blocks = order[: (N // n_cores) * n_cores].reshape(-1, n_cores)
        blocks[1::2] = blocks[1::2, ::-1]
        for c in range(n_cores):
            core_of[blocks[:, c]] = c
        # leftovers (none when N divisible)
        rest = order[(N // n_cores) * n_cores:]
        for i, n in enumerate(rest):
            core_of[n] = i % n_cores

        # ---- per-core greedy tile assignment balancing (tile, q) cells ----
        self.perm = []          # [T*P] global node id or -1
        self.slot_of = np.full(N, -1, np.int64)  # global -> tile*P+pos (local)
        maxcell = 0
        for c in range(n_cores):
            nodes = order[core_of[order] == c]
            cells = np.zeros((T, 4), np.int64)
            counts = np.zeros(T, np.int64)
            tile_nodes = [[] for _ in range(T)]
            for n in nodes:
                d4 = deg4[n]
                nm = np.where(counts < P, (cells + d4[None]).max(1), 1 << 40)
                t = int(np.argmin(nm))
                cells[t] += d4
                counts[t] += 1
                tile_nodes[t].append(n)
            maxcell = max(maxcell, int(cells.max()))
            pm = np.full(T * P, -1, np.int64)
            for t in range(T):
                tn = tile_nodes[t]
                pm[t * P: t * P + len(tn)] = tn
                for i, n in enumerate(tn):
                    self.slot_of[n] = t * P + i
            self.perm.append(pm)

        self.S = max(1, math.ceil(maxcell / P))
        S = self.S
        self.ncols = 4 * G * S
        assert self.ncols <= P, f"ncols={self.ncols} > 128"
        GS = G * S

        # ---- per-core edge slotting + gather/onehot arrays ----
        self.gidx = []   # [n_groups*16, 4*GS*8] int16
        self.gdl = []    # [n_groups*P, ncols]  bf16
        self.gsdT = []   # [n_groups*P, ncols*P] fp8
        fp8np = mybir.dt.np(FP8)
        core_of_dst = core_of[dst]
        for c in range(n_cores):
            sel = core_of_dst == c
            s_c = src[sel]
            dl = self.slot_of[dst[sel]]          # local slot id
            t_all = dl >> 7
            q_all = s_c // self.Qsz
            key = (t_all * 4 + q_all) * S        # segment -> first chunk slot
            ordr = np.argsort(key, kind="stable")
            s_c, dl, key = s_c[ordr], dl[ordr], key[ordr]
            segk = t_all[ordr] * 4 + q_all[ordr]
            cnt = np.bincount(segk, minlength=T * 4)
            assert cnt.max() <= S * P, (c, cnt.max(), S * P)
            starts = np.concatenate([[0], np.cumsum(cnt)])[:-1]
            pos_in_seg = np.arange(len(s_c)) - starts[segk]
            # flat layout: segment-major (t, q), inside: s*128+p
            flat_idx = np.zeros(T * 4 * S * P, np.int16)
            flat_dl = np.full(T * 4 * S * P, PAD_DST, np.float32)
            pos = segk * (S * P) + pos_in_seg
            flat_idx[pos] = (s_c - q_all[ordr] * self.Qsz).astype(np.int16)
            flat_dl[pos] = (dl & 127).astype(np.float32)

            fi = flat_idx.reshape(self.n_groups, G, 4, S, P)
            fd = flat_dl.reshape(self.n_groups, G, 4, S, P)
            # chunk order within a group: c = q*GS + t_loc*S + s
            fi = fi.transpose(0, 2, 1, 3, 4)   # [g, 4, G, S, P]
            fd = fd.transpose(0, 2, 1, 3, 4)

            # gather idx arrays: per (g, q), flat i=(t_loc*S+s)*128+p,
            # wrapped [16, i//16] (slot i -> partition i%16, col i//16)
            w = fi.reshape(self.n_groups, 4, GS * 8, 16)
            w = np.transpose(w, (0, 1, 3, 2))          # [g, 4, 16, GS*8]
            w = w.transpose(0, 2, 1, 3)                # [g, 16, 4, GS*8]
            w = np.tile(w, (1, 8, 1, 1))               # replicate to 128
            self.gidx.append(np.ascontiguousarray(
                w.reshape(self.n_groups * P, 4 * GS * 8)))

            # dlt [g, p(edge), ncols]
            fcol = fd.reshape(self.n_groups, self.ncols, P)
            gdl = np.transpose(fcol, (0, 2, 1))
            self.gdl.append(np.ascontiguousarray(
                gdl.reshape(self.n_groups * P, self.ncols)).astype(np.float32))

            # sdT [g, d, c*P+e] = (dl[c, e] == d)
            dli = fcol.astype(np.int32)                # [g, c, e]
            oh = (dli[:, None, :, :] ==
                  np.arange(P, dtype=np.int32)[None, :, None, None])
            self.gsdT.append(np.ascontiguousarray(
                oh.reshape(self.n_groups * P, self.ncols * P)
                .astype(NPBF if SDT_BF16 else fp8np)))


# --------------------------------------------------------------------------
# device kernel builder (shared by both layers)
# --------------------------------------------------------------------------

def build_layer_kernel(es: EdgeStruct, layer: int):
    """layer 1: rows [xh interleaved(128) | es(8)] bf16, ELEM 256, H=8;
       layer 2: rows [xh2(40) | es2(41) | one(42)] bf16, ELEM 128, H=1."""
    Npad, T, G, S, ncols = es.Npad, es.T, es.G, es.S, es.ncols
    n_groups, Qsz, GS = es.n_groups, es.Qsz, G * S
    if layer == 1:
        ELEM, WCOLS, WU, MW, H, AW, RW = 256, 136, 136, 128, 8, 136, 136
    else:
        ELEM, WCOLS, WU, MW, H, AW, RW = 128, 41, 42, 40, 1, 42, 42
    ES0 = MW            # es column start in table rows
    DEN0 = AW - H       # denominator columns in the accumulator
    CSZ = MW // H

    nc = bacc.Bacc("TRN2", target_bir_lowering=False, debug=False,
                   num_devices=es.n_cores)
    ap = {}

    def din(name, shape, dt):
        ap[name] = nc.dram_tensor(name, shape, dt, kind="ExternalInput").ap()

    din("xT", [P, Npad], BF16)
    din("xTm", [P, T * P], BF16)
    din("wext", [P, WCOLS], BF16)
    din("wdst", [P, H], BF16)
    din("drow", [1, H], BF16)
    din("iota_bf", [P, P], BF16)
    din("idn_bf", [P, P], BF16)
    din("ones_bf", [1, P], BF16)
    din("gidx", [n_groups * P, 4 * GS * 8], I16)
    din("gdl", [n_groups * P, ncols], F32)
    din("gsdT", [n_groups * P, ncols * P],
        BF16 if SDT_BF16 else FP8)
    if layer == 1:
        out_ap = nc.dram_tensor("hT", [P, T * P], BF16,
                                kind="ExternalOutput").ap()
    else:
        out_ap = nc.dram_tensor("logits", [T * P, MW], F32,
                                kind="ExternalOutput").ap()
    tbl = nc.dram_tensor("tbl", [Npad, ELEM], BF16, kind="Internal").ap()
    own_tbl = nc.dram_tensor("own_tbl", [T * P, ELEM], BF16,
                             kind="Internal").ap()

    with tile.TileContext(nc) as tc, ExitStack() as ctx:
        cpool = ctx.enter_context(tc.tile_pool(name="consts", bufs=1))
        wext = cpool.tile([P, WCOLS], BF16)
        nc.sync.dma_start(wext[:], ap["wext"])
        wdst = cpool.tile([P, H], BF16)
        nc.sync.dma_start(wdst[:], ap["wdst"])
        drow = cpool.tile([1, H], BF16)
        nc.sync.dma_start(drow[:], ap["drow"])
        iota_bf = cpool.tile([P, P], BF16)
        nc.sync.dma_start(iota_bf[:], ap["iota_bf"])
        idn_bf = cpool.tile([P, P], BF16)
        nc.sync.dma_start(idn_bf[:], ap["idn_bf"])
        ones_bf = cpool.tile([1, P], BF16)
        nc.sync.dma_start(ones_bf[:], ap["ones_bf"])
        edst_sb = cpool.tile([P, T * H], BF16)

        STB = 4  # staging bufs
        with tc.tile_pool(name="pre_sb", bufs=STB) as psb, \
                tc.tile_pool(name="pre_ps", bufs=2, space="PSUM") as pps:
            # staging tiles (L2: pre-set the constant-one column once per buf)
            stages = []
            for b in range(STB):
                st = psb.tile([P, 2, WU], BF16, tag="st")
                if WU > WCOLS:
                    nc.vector.memset(st[:, :, WCOLS:WU], 1.0)
                stages.append(st)

            def prepass(src_ap, n2, dst_tbl, emit_edst, base):
                for i in range(n2):
                    xt = psb.tile([P, 2 * P], BF16, tag="xt")
                    nc.sync.dma_start(xt[:], src_ap[:, i * 2 * P:(i + 1) * 2 * P])
                    pp = pps.tile([P, 2, WCOLS], F32, tag="pp")
                    for j in range(2):
                        nc.tensor.matmul(out=pp[:, j, :],
                                         lhsT=xt[:, j * P:(j + 1) * P],
                                         rhs=wext[:], start=True, stop=True,
                                         skip_group_check=True)
                    st = stages[i % STB]
                    eng = nc.vector if i % 2 == 0 else nc.scalar
                    if eng is nc.vector:
                        nc.vector.tensor_copy(out=st[:, :, 0:WCOLS], in_=pp[:])
                    else:
                        nc.scalar.activation(out=st[:, :, 0:WCOLS], in_=pp[:],
                                             func=AF.Copy)
                    dst = dst_tbl[i * 2 * P:(i + 1) * 2 * P, 0:WU] \
                        .rearrange("(j p) w -> p j w", p=P)
                    nc.sync.dma_start(dst, st[:])
                    if emit_edst:
                        pe = pps.tile([P, 2 * H], F32, tag="pe")
                        for j in range(2):
                            nc.tensor.matmul(out=pe[:, j * H:(j + 1) * H],
                                             lhsT=xt[:, j * P:(j + 1) * P],
                                             rhs=wdst[:], start=True,
                                             stop=False, skip_group_check=True)
                            nc.tensor.matmul(out=pe[:, j * H:(j + 1) * H],
                                             lhsT=ones_bf[:], rhs=drow[:],
                                             start=False, stop=True,
                                             skip_group_check=True)
                        nc.vector.tensor_copy(
                            out=edst_sb[:, i * 2 * H:(i + 1) * 2 * H],
                            in_=pe[:])

            prepass(ap["xT"], Npad // (2 * P), tbl, False, 0)
            prepass(ap["xTm"], T // 2, own_tbl, True, 0)

        # ---- edge pass ----
        ipool = ctx.enter_context(tc.tile_pool(name="ip", bufs=2))
        dpool = ctx.enter_context(tc.tile_pool(name="dp", bufs=2))
        spool = ctx.enter_context(tc.tile_pool(name="sp", bufs=2))
        gpool = ctx.enter_context(tc.tile_pool(name="gp", bufs=2))
        mpool = ctx.enter_context(tc.tile_pool(name="mp", bufs=2))
        apool = ctx.enter_context(tc.tile_pool(name="ap", bufs=2))
        wpool = ctx.enter_context(tc.tile_pool(name="wp", bufs=4))
        opool = ctx.enter_context(tc.tile_pool(name="op", bufs=2))
        epool = ctx.enter_context(tc.tile_pool(name="ep", bufs=2))
        pedp = ctx.enter_context(tc.tile_pool(name="pedp", bufs=2,
                                              space="PSUM"))
        pacc = ctx.enter_context(tc.tile_pool(name="pacc", bufs=G,
                                              space="PSUM"))
        ptp = ctx.enter_context(tc.tile_pool(name="ptp", bufs=2, space="PSUM"))

        for g in range(n_groups):
            idx = ipool.tile([P, 4 * GS * 8], I16, tag="idx")
            nc.sync.dma_start(idx[:], ap["gidx"][g * P:(g + 1) * P, :])
            dlt = dpool.tile([P, ncols], F32, tag="dl")
            nc.sync.dma_start(dlt[:], ap["gdl"][g * P:(g + 1) * P, :])
            sdt = spool.tile([P, ncols * P],
                             BF16 if SDT_BF16 else FP8, tag="sdt")
            nc.sync.dma_start(sdt[:], ap["gsdT"][g * P:(g + 1) * P, :])
            gb = gpool.tile([P, ncols * ELEM], BF16, tag="gb")
            gb3 = gb[:].rearrange("p (c k) -> p c k", k=ELEM)
            for q in range(4):
                for c0 in range(0, GS, MAXC):
                    cn = min(MAXC, GS - c0)
                    nc.gpsimd.dma_gather(
                        out_ap=gb3[:, q * GS + c0:q * GS + c0 + cn, :],
                        in_ap=tbl[q * Qsz:(q + 1) * Qsz, :],
                        idxs_ap=idx[:, (q * GS + c0) * 8:
                                    (q * GS + c0 + cn) * 8],
                        num_idxs=cn * P,
                        num_idxs_reg=cn * P,
                        elem_size=ELEM,
                    )
            # e_dst for every edge slot of the group (one psum tile)
            edp = pedp.tile([P, ncols * H], F32, tag="edp")
            for c in range(ncols):
                t = g * G + (c % GS) // S
                nc.tensor.matmul(out=edp[:, c * H:(c + 1) * H],
                                 lhsT=sdt[:, c * P:(c + 1) * P],
                                 rhs=edst_sb[:, t * H:(t + 1) * H],
                                 start=True, stop=True, skip_group_check=True)
            # attention scores, batched over the whole group
            al = apool.tile([P, ncols * H], F32, tag="al")
            al3 = al[:].rearrange("p (c h) -> p c h", h=H)
            edp3 = edp[:].rearrange("p (c h) -> p c h", h=H)
            nc.vector.tensor_tensor(out=al3, in0=gb3[:, :, ES0:ES0 + H],
                                    in1=edp3, op=ALU.add)
            alm = apool.tile([P, ncols * H], F32, tag="alm")
            nc.vector.tensor_scalar_mul(out=alm[:], in0=al[:],
                                        scalar1=NEG_SLOPE)
            nc.vector.tensor_tensor(out=al[:], in0=al[:], in1=alm[:],
                                    op=ALU.max)
            if layer == 1:
                mea = mpool.tile([P, ncols * AW], BF16, tag="mea")
                mea3 = mea[:].rearrange("p (c w) -> p c w", w=AW)
                nc.scalar.activation(out=mea3[:, :, MW:MW + H], in_=al[:],
                                     func=AF.Exp)
            else:
                ea2 = apool.tile([P, ncols], F32, tag="ea2")
                nc.scalar.activation(out=ea2[:], in_=al[:], func=AF.Exp)

            accs = []
            for t_loc in range(G):
                t = g * G + t_loc
                acc = pacc.tile([P, AW], F32, tag="acc")
                accs.append(acc)
                first = True
                for q in range(4):
                    for s in range(S):
                        c = q * GS + t_loc * S + s
                        last = (q == 3 and s == S - 1)
                        if layer == 1:
                            seT = wpool.tile([P, P], BF16, tag="seT")
                            nc.vector.tensor_scalar(
                                out=seT[:], in0=iota_bf[:],
                                scalar1=dlt[:, c:c + 1], scalar2=None,
                                op0=ALU.is_equal)
                            # msg = xh * ea (2x packed: interleaved heads)
                            xh4 = gb3[:, c, 0:MW].rearrange(
                                "p (cc h) -> p cc h", h=H)
                            ea4 = mea3[:, c:c + 1, MW:MW + H].to_broadcast(
                                [P, CSZ, H])
                            m4 = mea3[:, c, 0:MW].rearrange(
                                "p (cc h) -> p cc h", h=H)
                            nc.vector.tensor_tensor(out=m4, in0=xh4, in1=ea4,
                                                    op=ALU.mult)
                            nc.tensor.matmul(
                                out=acc[:],
                                lhsT=seT[:],
                                rhs=mea[:, c * AW:(c + 1) * AW],
                                start=first, stop=last,
                                skip_group_check=True)
                        else:
                            w = wpool.tile([P, P], BF16, tag="w")
                            nc.vector.tensor_scalar(
                                out=w[:], in0=iota_bf[:],
                                scalar1=dlt[:, c:c + 1],
                                scalar2=ea2[:, c:c + 1],
                                op0=ALU.is_equal, op1=ALU.mult)
                            nc.tensor.matmul(
                                out=acc[:], lhsT=w[:],
                                rhs=gb3[:, c, 0:RW],
                                start=first, stop=last,
                                skip_group_check=True)
                        first = False

            # ---- group epilogue (self-loops, softmax-div, out) ----
            own = opool.tile([P, G * WU], BF16, tag="own")
            own3 = own[:].rearrange("p (t w) -> p t w", w=WU)
            for t_loc in range(G):
                t = g * G + t_loc
                nc.sync.dma_start(
                    own3[:, t_loc, :],
                    own_tbl[t * P:(t + 1) * P, 0:WU])
            als = epool.tile([P, G * H], F32, tag="als")
            als3 = als[:].rearrange("p (t h) -> p t h", h=H)
            nc.vector.tensor_tensor(
                out=als3, in0=own3[:, :, ES0:ES0 + H],
                in1=edst_sb[:, g * G * H:(g + 1) * G * H]
                .rearrange("p (t h) -> p t h", h=H), op=ALU.add)
            alsm = epool.tile([P, G * H], F32, tag="alsm")
            nc.vector.tensor_scalar_mul(out=alsm[:], in0=als[:],
                                        scalar1=NEG_SLOPE)
            nc.vector.tensor_tensor(out=als[:], in0=als[:], in1=alsm[:],
                                    op=ALU.max)
            eas = epool.tile([P, G * H], F32, tag="eas")
            nc.scalar.activation(out=eas[:], in_=als[:], func=AF.Exp)
            eas3 = eas[:].rearrange("p (t h) -> p t h", h=H)
            if layer == 1:
                # own es cols -> 1.0 so sc = own*eas gives [own*eas | eas]
                nc.vector.memset(own3[:, :, ES0:ES0 + H], 1.0)
            sc = epool.tile([P, G * AW], F32, tag="sc")
            sc4 = sc[:].rearrange("p (t w h) -> p t w h", t=G, h=H)
            own4 = own3[:, :, 0:AW].rearrange("p t (w h) -> p t w h", h=H)
            nc.vector.tensor_tensor(
                out=sc4, in0=own4,
                in1=eas3.unsqueeze(2).to_broadcast([P, G, AW // H, H]),
                op=ALU.mult)
            stg = epool.tile([P, G * AW], F32, tag="stg")
            stg3 = stg[:].rearrange("p (t w) -> p t w", w=AW)
            sc3 = sc[:].rearrange("p (t w) -> p t w", w=AW)
            for t_loc in range(G):
                nc.vector.tensor_tensor(out=stg3[:, t_loc, :],
                                        in0=accs[t_loc][:],
                                        in1=sc3[:, t_loc, :], op=ALU.add)
            rec = epool.tile([P, G * H], F32, tag="rec")
            rec3 = rec[:].rearrange("p (t h) -> p t h", h=H)
            nc.vector.reciprocal(out=rec3, in_=stg3[:, :, DEN0:DEN0 + H])
            ot = epool.tile([P, G * MW], F32, tag="ot")
            ot4 = ot[:].rearrange("p (t w h) -> p t w h", t=G, h=H)
            nc.vector.tensor_tensor(
                out=ot4,
                in0=stg3[:, :, 0:MW].rearrange("p t (w h) -> p t w h", h=H),
                in1=rec3.unsqueeze(2).to_broadcast([P, G, CSZ, H]),
                op=ALU.mult)
            if layer == 1:
                # ELU then transpose out (bf16)
                otb = epool.tile([P, G * MW], BF16, tag="otb")
                nc.vector.tensor_copy(out=otb[:], in_=ot[:])
                t1 = epool.tile([P, G * MW], BF16, tag="t1")
                nc.vector.tensor_scalar_min(out=t1[:], in0=otb[:],
                                            scalar1=0.0)
                nc.scalar.activation(out=t1[:], in_=t1[:], func=AF.Exp)
                nc.vector.tensor_scalar_max(out=otb[:], in0=otb[:],
                                            scalar1=0.0)
                nc.vector.tensor_tensor(out=otb[:], in0=otb[:], in1=t1[:],
                                        op=ALU.add)
                nc.vector.tensor_scalar_add(out=otb[:], in0=otb[:],
                                            scalar1=-1.0)
                otb3 = otb[:].rearrange("p (t w) -> p t w", w=MW)
                for t_loc in range(G):
                    t = g * G + t_loc
                    tp = ptp.tile([P, P], BF16, tag="tp")
                    nc.tensor.transpose(out=tp[:], in_=otb3[:, t_loc, :],
                                        identity=idn_bf[:])
                    hTt = epool.tile([P, P], BF16, tag="hTt")
                    eng = nc.vector if t_loc % 2 == 0 else nc.scalar
                    if eng is nc.vector:
                        nc.vector.tensor_copy(out=hTt[:], in_=tp[:])
                    else:
                        nc.scalar.activation(out=hTt[:], in_=tp[:],
                                             func=AF.Copy)
                    nc.sync.dma_start(out_ap[:, t * P:(t + 1) * P], hTt[:])
            else:
                ot3 = ot[:].rearrange("p (t w) -> p t w", w=MW)
                for t_loc in range(G):
                    t = g * G + t_loc
                    nc.sync.dma_start(out_ap[t * P:(t + 1) * P, :],
                                      ot3[:, t_loc, :])

    nc.compile()
    return nc


# --------------------------------------------------------------------------
# host orchestration
# --------------------------------------------------------------------------

def _blockdiag(att):
    H, C = att.shape
    out = np.zeros((H * C, H), np.float32)
    for h in range(H):
        out[h * C:(h + 1) * C, h] = att[h]
    return out


def _consts():
    iota = np.arange(P, dtype=np.float32)
    return {
        "iota_bf": np.tile(iota.astype(NPBF)[None, :], (P, 1)),
        "idn_bf": np.eye(P, dtype=NPBF),
        "ones_bf": np.ones((1, P), NPBF),
    }


def _interleave_perm(H, C):
    # col j = cc*H + h  <- original h*C + cc
    j = np.arange(H * C)
    cc, h = j // H, j % H
    return h * C + cc


def _solve_shift(W, b):
    """c with c @ W == b (least squares; exact when b in rowspace)."""
    if not np.any(b):
        return np.zeros(W.shape[0], np.float32)
    c, *_ = np.linalg.lstsq(W.T.astype(np.float64),
                            b.astype(np.float64), rcond=None)
    return c.astype(np.float32)


def run_gat(x, edge_index, W1, att_src1, att_dst1, b1, W2, att_src2, att_dst2,
            b2, N, n_cores, es=None, verbose=False):
    x = np.asarray(x, np.float32)
    W1 = np.asarray(W1, np.float32)
    W2 = np.asarray(W2, np.float32)
    b1 = np.asarray(b1, np.float32)
    b2 = np.asarray(b2, np.float32)
    a_s1 = np.asarray(att_src1, np.float32)
    a_d1 = np.asarray(att_dst1, np.float32)
    a_s2 = np.asarray(att_src2, np.float32)
    a_d2 = np.asarray(att_dst2, np.float32)
    src = np.asarray(edge_index[0]).astype(np.int64)
    dst = np.asarray(edge_index[1]).astype(np.int64)

    if es is None:
        es = EdgeStruct(src, dst, N, n_cores)
    T, Npad = es.T, es.Npad
    consts = _consts()

    H1, C1 = a_s1.shape
    inter1 = _interleave_perm(H1, C1)
    asb1, adb1 = _blockdiag(a_s1), _blockdiag(a_d1)

    # layer-1 host matrices (bias folded via x-shift + e_dst correction)
    c1 = _solve_shift(W1, b1)
    wext1 = np.concatenate([W1[:, inter1], W1 @ asb1], 1).astype(NPBF)
    wdst1 = (W1 @ adb1).astype(NPBF)
    drow1 = (-(b1 @ asb1 + b1 @ adb1))[None, :].astype(NPBF)

    xs = x + c1[None, :]
    xT = np.zeros((P, Npad), NPBF)
    xT[:, :N] = xs.T.astype(NPBF)

    nc1 = build_layer_kernel(es, 1)
    in_maps = []
    for c in range(n_cores):
        pm = es.perm[c]
        valid = pm >= 0
        xTm = np.zeros((P, T * P), NPBF)
        xTm[:, valid] = xT[:, pm[valid]]
        in_maps.append({
            "xT": xT, "xTm": xTm, "wext": wext1, "wdst": wdst1,
            "drow": drow1, "gidx": es.gidx[c], "gdl": es.gdl[c],
            "gsdT": es.gsdT[c], **consts,
        })
    res1 = run_bass_kernel_spmd(nc1, in_maps, core_ids=list(range(n_cores)))

    # assemble global hidden table (features are (cc,h)-interleaved rows)
    hT = np.zeros((P, Npad), np.float32)
    for c in range(n_cores):
        pm = es.perm[c]
        valid = pm >= 0
        hT[:, pm[valid]] = np.asarray(
            res1.results[c]["hT"], np.float32)[:, valid]

    # layer-2 host matrices
    H2, C2 = a_s2.shape  # (1, 40)
    asb2, adb2 = _blockdiag(a_s2), _blockdiag(a_d2)
    c2 = _solve_shift(W2, b2)
    W2i = W2[inter1, :]
    wext2 = np.concatenate([W2i, (W2 @ asb2)[inter1]], 1).astype(NPBF)
    wdst2 = ((W2 @ adb2)[inter1]).astype(NPBF)
    drow2 = (-(b2 @ asb2 + b2 @ adb2))[None, :].astype(NPBF)

    hTs = hT + c2[inter1][:, None]
    hTb = hTs.astype(NPBF)
    nc2 = build_layer_kernel(es, 2)
    in_maps2 = []
    for c in range(n_cores):
        pm = es.perm[c]
        valid = pm >= 0
        hTm = np.zeros((P, T * P), NPBF)
        hTm[:, valid] = hTb[:, pm[valid]]
        in_maps2.append({
            "xT": hTb, "xTm": hTm, "wext": wext2, "wdst": wdst2,
            "drow": drow2, "gidx": es.gidx[c], "gdl": es.gdl[c],
            "gsdT": es.gsdT[c], **consts,
        })
    res2 = run_bass_kernel_spmd(nc2, in_maps2, core_ids=list(range(n_cores)))

    out = np.zeros((N, W2.shape[1]), np.float32)
    for c in range(n_cores):
        pm = es.perm[c]
        valid = pm >= 0
        out[pm[valid]] = np.asarray(
            res2.results[c]["logits"], np.float32)[valid, :]
    return out


def kernel(x, edge_index, W1, att_src1, att_dst1, b1, W2, att_src2, att_dst2,
           b2):
    N = int(np.asarray(x).shape[0])
    return run_gat(x, edge_index, W1, att_src1, att_dst1, b1, W2, att_src2,
                   att_dst2, b2, N=N, n_cores=8)
